# revision 34
# baseline (speedup 1.0000x reference)
"""Trainium2 Bass kernel for BrickVectorEdgeModel (GNN message passing).

Reference computation (per batch element b of 8):
  f  = relu(relu(x @ Wa + ba) @ Wb + bb)            # node MLP, x: [128, 256]
  e[i, j] = cat(f[j], f[i])                         # pairwise concat
  h1 = relu(e @ Wca + bca)                          # decomposed: G[j] + H[i]
  h2 = relu(h1 @ Wcb + bcb)
  h3 = relu(h2 @ Wcc + bcc)
  out[i, j] = h3 @ Wo + bo                          # [128, 128, 2]

Sharding: data-parallel over batch, one batch element per NeuronCore (8 cores).

Device kernel works in transposed activation layout [feat (partitions), cols]:
each layer is out_T[fo, col] = sum_k W[k, fo] * act_T[k, col], i.e.
matmul(psum, lhsT=W_chunk, rhs=actT_chunk), so activations never need an
on-chip transpose. The first edge layer is decomposed:
  h1_T[:, (i, j)] = relu(G_T[:, j] + (H_T[:, i] + bca))
which is a per-partition-scalar broadcast add + relu (one tensor_scalar op
per 128x128 block) instead of a [16384, 1024] x [1024, 512] matmul.

All matmuls run in bf16 with fp32 PSUM accumulation.

Performance structure (~266 us vs the ~283 us v1 baseline):
- Output layer (M=2) runs as 4 col-tiled concurrent matmuls at
  tile_position (0, 32c), one per k-chunk, into a dedicated PSUM bank;
  the [2, 512] partials are ACT-copied wide to a 4-group staging tile,
  DMA'd out in batches, and summed (plus bo) on the host. This replaces
  4 sequential N=512 matmuls (864 ns PE) with ~1 matmul span per group.
- cb/cc consume their k-chunks in natural order: with the out quartet
  delaying cc by only ~430 ns, h2 chunk 3 is drain-ready just before its
  slot-3 use; a rotated order consumed it a slot early (guaranteed stall).
- Engine assignment (measured best): h2 drains + out copy on ACT,
  h3 drains + h1 builds on DVE, GpSimd unused (its tensor_scalar is 2x
  worse end-to-end), weight loads packed into 5 need-ordered DMAs on
  the sync queue (cb's chunk alone rides the idle scalar queue in
  parallel), 5 warm-up matmuls trip the HAM clock gate during the
  DMA-bound start, and the final output DMAs fan out across queues.
"""

import numpy as np
import ml_dtypes

import concourse.bass as bass
import concourse.mybir as mybir
import concourse.tile as tile
from concourse import bacc
from concourse.bass_utils import run_bass_kernel_spmd

BF16 = mybir.dt.bfloat16
F32 = mybir.dt.float32

B = 8          # batch == number of cores
N = 128        # bricks per model (nodes)
D_IN = 256     # input feature dim
H = 512        # hidden dim
KA = D_IN // 128   # 2 input-feature chunks
C = H // 128       # 4 hidden-feature chunks
IG = 4             # i-values per group (4 * 128 cols = 512 = one PSUM bank)
NG = N // IG       # 32 groups

# Stashed by kernel() for harnesses that want profiling info (exec_time_ns
# is populated when BASS_TRACE=1 and the NTFF hook is available).
LAST_RESULTS = None


def _build_nc() -> bass.Bass:
    # Bacc (not raw Bass): its compile pass legalizes multi-wait sync_info
    # into forms walrus codegen accepts (raw Bass + Tile hits "Too many
    # sync wait commands" in CoreV2GenImpl setupSyncWait).
    nc = bacc.Bacc("TRN2", target_bir_lowering=False)

    # Inputs (host pre-packs: weights [K, F] -> [128, K//128, F] bf16,
    # biases [F] -> [128, F//128] f32, x -> x.T packed the same way).
    # One packed bf16 weight buffer (column offsets below) + one packed f32
    # bias buffer + per-core xT. Packing cuts the load-DMA count from 17 to
    # 6 — each dma_start costs ~640 ns *issue time* serialized on the queue,
    # which (not HBM bandwidth) gates how early the edge weights land.
    WA_O = 0                    # [KA, H]
    WB_O = WA_O + KA * H        # [C, H]
    WCAJ_O = WB_O + C * H       # [C, H]
    WCAI_O = WCAJ_O + C * H     # [C, H]
    WCB_O = WCAI_O + C * H      # [C, H]
    WCC_O = WCB_O + C * H       # [C, H]
    WO_O = WCC_O + C * H        # [C, 2]
    WP_COLS = WO_O + C * 2
    xT = nc.dram_tensor("xT", [128, KA, N], BF16, kind="ExternalInput")
    Wp = nc.dram_tensor("Wp", [128, WP_COLS], BF16, kind="ExternalInput")
    Bp = nc.dram_tensor("Bp", [128, 5 * C], F32, kind="ExternalInput")

    # Output: per 4-group batch gb and k-chunk c, the [2, 4, IG, N] partials
    # of the (unbiased) output layer. Host sums over c and adds bo.
    outP = nc.dram_tensor("outP", [NG // 4, C, 2, 4, IG, N], F32,
                          kind="ExternalOutput")
    GB = 4  # groups per output DMA batch

    relu = mybir.ActivationFunctionType.Relu
    ident = mybir.ActivationFunctionType.Identity
    add_op = mybir.AluOpType.add
    max_op = mybir.AluOpType.max

    with tile.TileContext(nc) as tc:
        with (
            tc.tile_pool(name="consts", bufs=1) as consts,
            tc.tile_pool(name="work", bufs=4) as work,
            tc.tile_pool(name="outp", bufs=2) as outp,
            tc.tile_pool(name="psmid", bufs=7, space="PSUM") as psmid,
            tc.tile_pool(name="psout", bufs=1, space="PSUM") as psout,
        ):
            # ---- PE warm-up -----------------------------------------------------
            # The HAM clock gate holds the PE at 1.2 GHz until it has seen
            # ~3.4 us of sustained activity. v12 trace: K=8/8 fired only at
            # 20.2us — the 5 warm-ups (memset'd on the DVE, which is booting
            # until ~7.4us) started at 8.4us and were too short a burst.
            # Now: memset on GpSimd (ready ~5.8us, earliest engine) and a
            # longer burst sized to bridge to the first real matmul (~10us),
            # so the gate trips ~3.4us into the burst and the node/GH phase
            # runs at 2.4 GHz.
            warm_sb = consts.tile([128, H], BF16, tag="warm_sb")
            nc.gpsimd.memset(warm_sb, 0.0)
            warm_ps = psmid.tile([128, IG * N], F32, tag="pst")
            for _ in range(9):
                nc.tensor.matmul(warm_ps, warm_sb[:, 0:128], warm_sb,
                                 start=True, stop=True)

            # Dummy matmuls emitted at known PE-bubble points (DMA/drain
            # waits in the node/GH phase). They must have NO deps so they
            # run during the wait and keep the HAM activity window busy —
            # v13 trace: the gate re-throttled to 1.2 GHz at 14.5us because
            # the node-phase bubbles left the window ~50% idle, and groups
            # 0-1 of the edge loop then ran cold. v15 trace: fills drawn
            # from psmid inherit WAR deps on the node-layer ACT drains via
            # slot recycling and stall the in-order PE queue — so they live
            # in the psout bank, which is idle until group 0's quartet
            # (same tag as pso so the 1-buf pool doesn't double-book).
            fill_ps = psout.tile([128, IG * N], F32, tag="pso")

            def pe_fill(n, cols=256):
                for _ in range(n):
                    nc.tensor.matmul(fill_ps[:, 0:cols], warm_sb[:, 0:128],
                                     warm_sb[:, 0:cols], start=True, stop=True)

            # ---- load constants -------------------------------------------------
            # All loads on the sync queue in need-order, with Wcb alone on
            # the scalar queue. Measured alternatives that LOSE: a balanced
            # sync/scalar split (v19: both queues drop to ~130 GB/s — the
            # ~260 GB/s is an HBM-side per-core cap, not per-queue — and
            # everything lands LATER: 19.7us vs 17.8us), and issue ops on
            # the scalar ENGINE delay its drain chain ~600ns each. Also
            # v5: gpsimd/vector queues boot ~6.7us late.
            xT_sb = consts.tile([128, KA, N], BF16, tag="xT_sb")
            nc.sync.dma_start(out=xT_sb, in_=xT[:])
            bp_sb = consts.tile([128, 5 * C], F32, tag="bp_sb")
            nc.sync.dma_start(out=bp_sb, in_=Bp[:])
            wp_sb = consts.tile([128, WP_COLS], BF16, tag="wp_sb")
            cuts = [WA_O, WB_O, WCAJ_O, WCB_O, WCC_O, WP_COLS]
            for s in range(len(cuts) - 1):
                q = nc.scalar if cuts[s] == WCB_O else nc.sync
                q.dma_start(out=wp_sb[:, cuts[s]:cuts[s + 1]],
                            in_=Wp[:, cuts[s]:cuts[s + 1]])

            def wv(off, k, fo, w=128):
                return wp_sb[:, off + k * H + fo * w:off + k * H + (fo + 1) * w]

            ba_sb = bp_sb[:, 0 * C:1 * C]
            bb_sb = bp_sb[:, 1 * C:2 * C]
            bca_sb = bp_sb[:, 2 * C:3 * C]
            bcb_sb = bp_sb[:, 3 * C:4 * C]
            bcc_sb = bp_sb[:, 4 * C:5 * C]

            # ---- node MLP (tiny): f2_T = relu(Wb_T @ relu(Wa_T @ x_T + ba) + bb)
            # Each layer's 4 fo-chunks go to disjoint 128-col slices of ONE
            # psum bank. Drains split ACT/DVE by fo parity: v16 trace shows
            # the whole startup gated by a single serialized ACT chain
            # (~780ns per drain incl. sem overhead) — the GH matmuls and
            # the h1 g0 builds all queue behind it, delaying the edge
            # stream and re-throttling the HAM clock.
            def drain(out_ap, in_ap, bias_ap, fo):
                if fo % 2 == 0:
                    nc.scalar.activation(out_ap, in_ap, relu, bias=bias_ap)
                else:
                    nc.vector.tensor_scalar(out_ap, in_ap, bias_ap, 0.0,
                                            add_op, max_op)

            def node_layer(w_off, in_sb, kc, out_sb, bias_sb):
                pst = psmid.tile([128, C, N], F32, tag="pst")
                for fo in range(C):
                    for k in range(kc):
                        nc.tensor.matmul(
                            pst[:, fo, :], wv(w_off, k, fo),
                            in_sb[:, k, :],
                            start=(k == 0), stop=(k == kc - 1),
                        )
                    drain(out_sb[:, fo, :], pst[:, fo, :],
                          bias_sb[:, fo:fo + 1], fo)

            f1_sb = consts.tile([128, C, N], BF16, tag="f1_sb")
            node_layer(WA_O, xT_sb, KA, f1_sb, ba_sb)
            pe_fill(3)
            f2_sb = consts.tile([128, C, N], BF16, tag="f2_sb")
            node_layer(WB_O, f1_sb, C, f2_sb, bb_sb)
            pe_fill(3)

            # ---- G_T = Wcaj_T @ f2_T ; H'_T = Wcai_T @ f2_T + bca --------------
            # Chunk-interleaved with per-chunk drains and the group-0 h1 build
            # so the first edge matmul isn't gated on the full G/H production.
            gt_sb = consts.tile([128, C, N], BF16, tag="gt_sb")
            ht_sb = consts.tile([128, C, N], F32, tag="ht_sb")
            h1_first = work.tile([128, C, IG * N], BF16, tag="h1_sb")

            # One h1-build block: out = relu(G_T[c][:, j] + H'_T[c][:, hcol]).
            # il 0-2 on DVE (~250ns each), il 3 on ACT (~390ns) — v17
            # trace: DVE carried 16 builds + 4 h3 drains = 7.0us per 7.31us
            # group period (95% busy), serializing the early stream, while
            # ACT sat at 47%. GpSimd is NOT usable: its tensor_scalar on
            # [128,128] measures 2051ns (8x DVE) and v18 made it the global
            # bottleneck (356us).
            def build_one(out_ap, c, hcol, il):
                if il < 3:
                    nc.vector.tensor_scalar(
                        out_ap, gt_sb[:, c, :], ht_sb[:, c, hcol:hcol + 1],
                        0.0, add_op, max_op)
                else:
                    nc.scalar.activation(
                        out_ap, gt_sb[:, c, :], relu,
                        bias=ht_sb[:, c, hcol:hcol + 1])
            # One packed psum bank for all G chunks and one for all H (like
            # node_layer): per-fo psmid.tile allocations put 8 tiles through
            # the 7-slot ring, and the resulting WAR couplings stalled the
            # PE ~0.45us per fo (v19-21 traces).
            pstG = psmid.tile([128, C, N], F32, tag="pst")
            pstH = psmid.tile([128, C, N], F32, tag="pst")
            for fo in range(C):
                for k in range(C):
                    nc.tensor.matmul(
                        pstG[:, fo, :], wv(WCAJ_O, k, fo), f2_sb[:, k, :],
                        start=(k == 0), stop=(k == C - 1),
                    )
                if fo % 2 == 0:
                    nc.vector.tensor_copy(gt_sb[:, fo, :], pstG[:, fo, :])
                else:
                    nc.scalar.copy(gt_sb[:, fo, :], pstG[:, fo, :])
                for k in range(C):
                    nc.tensor.matmul(
                        pstH[:, fo, :], wv(WCAI_O, k, fo), f2_sb[:, k, :],
                        start=(k == 0), stop=(k == C - 1),
                    )
                if fo % 2 == 0:
                    nc.scalar.activation(ht_sb[:, fo, :], pstH[:, fo, :],
                                         ident, bias=bca_sb[:, fo:fo + 1])
                else:
                    nc.vector.tensor_scalar_add(ht_sb[:, fo, :],
                                                pstH[:, fo, :],
                                                bca_sb[:, fo:fo + 1])
                for il in range(IG):
                    build_one(h1_first[:, fo, il * N:(il + 1) * N],
                              fo, il, il)
                pe_fill(4)
            pe_fill(6)

            # ---- edge MLP over 32 groups of 4 i-values (512 cols each) ---------
            # h1-build for group g+1 is emitted mid-body (before group g's h3
            # drains) so the DVE FIFO runs it while the PE works on group g —
            # the next group's cb matmuls then start without waiting on DVE.
            def build_h1(g):
                # h1[c][:, il*128 + j] = relu(G_T[c][:, j] + H'_T[c][:, g*IG+il])
                # il 0,1 on DVE, il 2 on ACT, il 3 on GpSimd (see build_one;
                # giving GpSimd HALF the builds measured 2x worse overall —
                # a quarter keeps it at ~30% busy).
                t = work.tile([128, C, IG * N], BF16, tag="h1_sb")
                for c in range(C):
                    for il in range(IG):
                        build_one(t[:, c, il * N:(il + 1) * N],
                                  c, g * IG + il, il)
                return t

            # Output layer for group g: 4 col-tiled concurrent M=2 matmuls
            # (k-chunk c at columns 32c..32c+1 of one PSUM bank), then a
            # PSUM->SBUF copy (ACT) into a 4-group staging tile; every 4th
            # group the [2, 4, IG, N] partials are DMA'd out (batched to
            # keep the sync queue + epilogue short). The host sums partials
            # over c and adds bo. Emitted AFTER the next group's cb matmuls
            # so the PE never idles at a group boundary waiting for h3
            # drains. Tail: groups {28,29,30} ship at g=30 and group 31
            # DMAs straight out of PSUM (no staging copy), so only 4x4KB
            # slivers remain after the final quartet.

            def emit_out(g, h3_sb, state={}):
                pso = psout.tile([128, IG, N], F32, tag="pso")
                for c in range(C):
                    nc.tensor.matmul(
                        pso[32 * c:32 * c + 2],
                        wp_sb[:, WO_O + 2 * c:WO_O + 2 * c + 2], h3_sb[:, c, :],
                        start=True, stop=True, tile_position=(0, 32 * c),
                    )
                if g == NG - 1:
                    # One full-tile DVE copy (a partition-sliced copy costs
                    # the same ~680ns — per-lane rate, and only 2 lanes are
                    # live), then the 4KB slivers split across both booted
                    # queues. DVE is idle here (no next-group h1 build);
                    # ACT is still draining this group's h2/h3 chunks.
                    o_last = outp.tile([128, IG, N], F32, tag="o_last")
                    nc.vector.tensor_copy(o_last, pso)
                    for c in range(C):
                        q = nc.sync if c < 2 else nc.scalar
                        q.dma_start(out=outP[g // GB, c, :, GB - 1],
                                    in_=o_last[32 * c:32 * c + 2])
                    return
                if g % GB == 0:
                    o_new = outp.tile([128, GB, IG, N], F32, tag="o_sb")
                    state["o_sb"] = o_new
                o_sb = state["o_sb"]
                nc.scalar.copy(o_sb[:, g % GB], pso)
                if g == NG - 2:
                    for c in range(C):
                        nc.sync.dma_start(out=outP[g // GB, c, :, 0:GB - 1],
                                          in_=o_sb[32 * c:32 * c + 2, 0:GB - 1])
                elif g % GB == GB - 1:
                    for c in range(C):
                        nc.sync.dma_start(out=outP[g // GB, c],
                                          in_=o_sb[32 * c:32 * c + 2])

            h1_next = h1_first
            h3_prev = None
            for g in range(NG):
                h1_sb = h1_next
                # Emit the next group's h1 build first: the DVE starts it
                # immediately (it has no deps on group g), keeping its work
                # out of the contended cc-phase window.
                if g + 1 < NG:
                    h1_next = build_h1(g + 1)

                # h2 = relu(Wcb_T @ h1 + bcb). Natural k-order: with the
                # col-tiled out quartet delaying cc by only ~430 ns, chunk 3
                # of h2 is drain-ready just before its slot-3 use; the old
                # per-fo rotation consumed it at slot 2, a guaranteed stall.
                h2_sb = work.tile([128, C, IG * N], BF16, tag="h2_sb")
                for fo in range(C):
                    pst = psmid.tile([128, IG * N], F32, tag="pst")
                    ks = list(range(C))
                    for idx, k in enumerate(ks):
                        nc.tensor.matmul(
                            pst, wv(WCB_O, k, fo), h1_sb[:, k, :],
                            start=(idx == 0), stop=(idx == C - 1),
                        )
                    nc.scalar.activation(h2_sb[:, fo, :], pst, relu,
                                         bias=bcb_sb[:, fo:fo + 1])

                if h3_prev is not None:
                    emit_out(g - 1, h3_prev)

                # h3 = relu(Wcc_T @ h2 + bcc); natural k-order (see above).
                # Drains: fo0 on ACT, fo1-3 on DVE — rebalances the
                # 95%-busy DVE (see build_one) to ~72% with ACT at ~78%.
                h3_sb = work.tile([128, C, IG * N], BF16, tag="h3_sb")
                for fo in range(C):
                    pst = psmid.tile([128, IG * N], F32, tag="pst")
                    ks = list(range(C))
                    for idx, k in enumerate(ks):
                        nc.tensor.matmul(
                            pst, wv(WCC_O, k, fo), h2_sb[:, k, :],
                            start=(idx == 0), stop=(idx == C - 1),
                        )
                    if fo == 0:
                        nc.scalar.activation(h3_sb[:, fo, :], pst, relu,
                                             bias=bcc_sb[:, fo:fo + 1])
                    else:
                        nc.vector.tensor_scalar(
                            h3_sb[:, fo, :], pst, bcc_sb[:, fo:fo + 1], 0.0,
                            add_op, max_op,
                        )
                h3_prev = h3_sb

            emit_out(NG - 1, h3_prev)

    nc.compile()
    return nc


def _pack_w(w: np.ndarray) -> np.ndarray:
    """[K, F] f32 -> [128, K//128, F] bf16 so W[k, f] = packed[k % 128, k // 128, f]."""
    k, f = w.shape
    return np.ascontiguousarray(
        w.reshape(k // 128, 128, f).transpose(1, 0, 2)
    ).astype(ml_dtypes.bfloat16)


def _pack_b(b: np.ndarray) -> np.ndarray:
    """[F] f32 -> [128, F//128] f32 so b[f] = packed[f % 128, f // 128]."""
    return np.ascontiguousarray(b.reshape(-1, 128).T).astype(np.float32)


def kernel(brick_vectors, Wa, ba, Wb, bb, Wca, bca, Wcb, bcb, Wcc, bcc, Wo, bo):
    global LAST_RESULTS
    brick_vectors = np.asarray(brick_vectors, dtype=np.float32)

    wp = np.concatenate([
        _pack_w(np.asarray(Wa)).reshape(128, -1),
        _pack_w(np.asarray(Wb)).reshape(128, -1),
        _pack_w(np.asarray(Wca)[:H]).reshape(128, -1),
        _pack_w(np.asarray(Wca)[H:]).reshape(128, -1),
        _pack_w(np.asarray(Wcb)).reshape(128, -1),
        _pack_w(np.asarray(Wcc)).reshape(128, -1),
        _pack_w(np.asarray(Wo)).reshape(128, -1),
    ], axis=1)
    bp = np.concatenate([
        _pack_b(np.asarray(b)) for b in (ba, bb, bca, bcb, bcc)
    ], axis=1)
    shared = {"Wp": np.ascontiguousarray(wp), "Bp": np.ascontiguousarray(bp)}

    in_maps = []
    for b in range(B):
        xt = _pack_w(brick_vectors[b].T.astype(np.float32))  # [128, KA, N] bf16
        in_maps.append({"xT": xt, **shared})

    nc = _build_nc()
    res = run_bass_kernel_spmd(nc, in_maps, core_ids=list(range(B)))
    LAST_RESULTS = res

    bo_f = np.asarray(bo, dtype=np.float32).reshape(1, 2, 1, 1, 1)
    out = np.empty((B, N, N, 2), dtype=np.float32)
    for b in range(B):
        p = res.results[b]["outP"]               # [NG/4, C, 2, 4, IG, N]
        ot = p.sum(axis=1) + bo_f                # [NG/4, 2, 4, IG, N]
        out[b] = ot.transpose(0, 2, 3, 4, 1).reshape(N, N, 2)
    return out

